# revision 38
# baseline (speedup 1.0000x reference)
"""HardNegTripletMarginLoss on 8 Trainium2 NeuronCores (Bass/Tile).

Strategy (anchors row-sharded across 8 cores, embeddings replicated):
  - Host: normalize rows (as reference), stable-sort rows by label, and give
    each core a column-ROTATED copy of Xn^T so the core's own anchor block
    sits at local columns [0, 1024). With sorted labels every anchor's
    same-label columns then fall inside 2 statically-known 512-wide column
    tiles per anchor block -- identical tile indices on every core, so one
    SPMD program serves all 8 cores.
  - Device (per 128-anchor block, 4 PSUM chunks of 2048 = 16 col-tiles):
    PSUM tile = -2*G via bf16 matmuls (full PE rate) and, on the two
    "masked" tiles per block, += 8*same via a one-hot (K=64) matmul.
    Chunk c1 is min-reduced directly from PSUM on the Vector engine;
    chunks c0/c2/c3 are copied PSUM->SBUF as fp16 by the Scalar engine and
    min-reduced on Vector with a 2x-mode tensor_tensor tree. The two masked
    tiles (always inside c0/c3) get a fused max pair + reduce for the
    hardest-positive.  d2 = s_i + s_j - 2G with s_j ~= 1; s_i applied
    exactly via a per-partition bias before sqrt.
  - Host: gather per-anchor losses, loss = sum(per)/count(per>0).

This walrus build rejects instructions carrying >1 sync wait, so
Bass.to_json_bytes is wrapped to split multi-wait instructions into
single-wait Drain carriers on the same engine.
"""

import json
import os
import sys
import types
import ctypes

for _p in ("/opt/trn_rl_repo", "/root/.axon_site/_ro/trn_rl_repo"):
    if os.path.isdir(_p) and _p not in sys.path:
        sys.path.append(_p)

import numpy as np
import ml_dtypes
import concourse.bass as bass
import concourse.tile as tile
from concourse import mybir
from concourse.bass_utils import run_bass_kernel_spmd
from contextlib import ExitStack

P = 128
N = 8192
D = 128
NCORES = 8
M = N // NCORES            # anchors per core
NBLK = M // P              # anchor blocks per core
TW = 512                   # matmul moving-dim tile
CW = 2048                  # PSUM chunk (4 banks)
NCH = N // CW
TPC = CW // TW
BIG = 8.0
MARGIN = 0.05
F32 = mybir.dt.float32
F16 = mybir.dt.float16
BF16 = mybir.dt.bfloat16
BF = ml_dtypes.bfloat16

# masked (diagonal-window) global col-tile indices per anchor block l
MASKED_TILES = {0: (15, 0), 1: (15, 0), 2: (0, 1), 3: (0, 1),
                4: (0, 1), 5: (0, 1), 6: (1, 2), 7: (1, 2)}
# column slab layout of the one-hot cols input: tile -> slab slot
OH_SLOT = {15: 0, 0: 1, 1: 2, 2: 3}
# fp16-copy slice (chunk, offset) for each maskable tile
CP_SLICE = {15: (3, 1536), 0: (0, 0), 1: (0, 512), 2: (0, 1024)}

LAST_RESULTS = None        # BassKernelResults of the most recent run (for test.py)


def _install_wait_split_patch():
    if getattr(bass.Bass, "_wait_split_patched", False):
        return
    orig = bass.Bass.to_json_bytes

    def patched(self):
        raw = orig(self)
        d = json.loads(raw)
        changed = False
        for fn in d.get("functions", []):
            for blk in fn.get("blocks", []):
                out, k = [], 0
                for ins in blk.get("instructions", []):
                    si = ins.get("sync_info") or {}
                    waits = si.get("on_wait") or []
                    if len(waits) > 1:
                        changed = True
                        for w in waits[:-1]:
                            k += 1
                            out.append({
                                "name": f"{ins['name']}-sw{k}",
                                "opcode": "Drain",
                                "engine": ins["engine"],
                                "ins": [],
                                "outs": [],
                                "is_reset_sema": False,
                                "debug": ins.get("debug", 0),
                                "sync_info": {"on_wait": [w], "on_update": []},
                            })
                        si["on_wait"] = [waits[-1]]
                    out.append(ins)
                blk["instructions"] = out
        return json.dumps(d).encode() if changed else raw

    bass.Bass.to_json_bytes = patched
    bass.Bass._wait_split_patched = True


def _ensure_ntff_hook():
    """Best-effort: restore the axon NTFF profile hook this image dropped."""
    if "antenv.axon_hooks" in sys.modules:
        return
    try:
        lib = ctypes.CDLL("/opt/axon/libaxon_pjrt.so")
        if not hasattr(lib, "axon_start_nrt_profile"):
            return
        from trn_agent_boot.trn_boot import _ntff_profile_via_ctypes
        hook = _ntff_profile_via_ctypes("/opt/axon/libaxon_pjrt.so")
        mod = types.ModuleType("antenv.axon_hooks")
        mod._hook = hook
        mod.get_axon_ntff_profile_hook = lambda: mod._hook
        mod.set_axon_ntff_profile_hook = lambda h: setattr(mod, "_hook", h)
        sys.modules["antenv.axon_hooks"] = mod
        import antenv
        antenv.axon_hooks = mod
    except Exception:
        pass


def _build_nc():
    nc = bass.Bass("TRN2", target_bir_lowering=False, debug=False)
    xt_d = nc.dram_tensor("xt", [P, N], BF16, kind="ExternalInput")
    xa_d = nc.dram_tensor("xm2a", [P, M], BF16, kind="ExternalInput")
    ohb_d = nc.dram_tensor("ohb", [64, M + 4 * TW], BF16, kind="ExternalInput")
    bias_d = nc.dram_tensor("bias16", [P, 2 * NBLK], F32, kind="ExternalInput")
    out_d = nc.dram_tensor("per_out", [P, NBLK], F32, kind="ExternalOutput")

    MIN = mybir.AluOpType.min
    MAX = mybir.AluOpType.max
    AXX = mybir.AxisListType.X

    with tile.TileContext(nc) as tc, ExitStack() as ctx:
        xpool = ctx.enter_context(tc.tile_pool(name="xt", bufs=NCH))
        inpool = ctx.enter_context(tc.tile_pool(name="ins", bufs=1))
        cpool = ctx.enter_context(tc.tile_pool(name="cp", bufs=24))
        t1pool = ctx.enter_context(tc.tile_pool(name="t1", bufs=4))
        t2pool = ctx.enter_context(tc.tile_pool(name="t2", bufs=16))
        t3pool = ctx.enter_context(tc.tile_pool(name="t3", bufs=16))
        mxpool = ctx.enter_context(tc.tile_pool(name="mx", bufs=8))
        ppool = ctx.enter_context(tc.tile_pool(name="psum", bufs=2, space="PSUM"))
        accpool = ctx.enter_context(tc.tile_pool(name="acc", bufs=1))
        fpool = ctx.enter_context(tc.tile_pool(name="fin", bufs=4))

        # DMA order: xa + first xt chunk first so block 0's matmuls start as
        # early as possible; remaining xt chunks follow the small inputs.
        xa = inpool.tile([P, M], BF16, tag="xa")
        nc.sync.dma_start(xa[:], xa_d.ap()[:, :])
        xch = []
        for _i in range(NCH):
            _t = xpool.tile([P, CW], BF16, tag="xch", name=f"xch{_i}")
            xch.append(_t)
        nc.sync.dma_start(xch[0][:], xt_d.ap()[:, 0:CW])
        ohb = inpool.tile([64, M + 4 * TW], BF16, tag="ohb")
        nc.sync.dma_start(ohb[:], ohb_d.ap()[:, :])
        oha = ohb[:, 0:M]
        ohc = ohb[:, M:M + 4 * TW]
        bias16 = inpool.tile([P, 2 * NBLK], F32, tag="bias16")
        nc.sync.dma_start(bias16[:], bias_d.ap()[:, :])
        for ch in range(1, NCH):
            nc.sync.dma_start(xch[ch][:], xt_d.ap()[:, ch * CW:(ch + 1) * CW])

        # minb[:, l] = direct min of chunk c1; minb[:, NBLK+l] = tree min of
        # c0/c2/c3.  fin2 = [merged min | masked max] -- maxes land in the
        # high half so the finale needs no gather.
        minb = accpool.tile([P, 2 * NBLK], F32, tag="minb", name="minb")
        fin2 = accpool.tile([P, 2 * NBLK], F32, tag="fin2", name="fin2")

        # PE pre-warm: matmuls on a memset garbage tile -- no DMA dependency,
        # so they run during the input DMA wait and flip the HAM clock-gate to
        # 2.4 GHz before the first real matmul.
        junk = inpool.tile([P, TW], BF16, tag="junk")
        nc.vector.memset(junk[:], 1.0)
        warm_ps = ppool.tile([P, CW], F32, tag="ps")
        for _ in range(36):
            nc.tensor.matmul(warm_ps[:, 0:P], lhsT=junk[:, 0:P], rhs=junk[:, 0:P],
                             start=True, stop=True)

        def emit_block_mms(l, direct_ch=1, emit_direct=True):
            """PE matmuls + PSUM drains (ACT copies / DVE direct reduce)."""
            lhsT = xa[:, l * P:(l + 1) * P]
            cps = {}
            for ch in range(NCH):
                ps = ppool.tile([P, CW], F32, tag="ps")
                masked_us = []
                for u in range(TPC):
                    tg = ch * TPC + u
                    m = tg in MASKED_TILES[l]
                    if m:
                        masked_us.append((u, tg))
                    nc.tensor.matmul(
                        ps[:, u * TW:(u + 1) * TW],
                        lhsT=lhsT,
                        rhs=xch[ch][:, u * TW:(u + 1) * TW],
                        start=True, stop=not m)
                for u, tg in masked_us:
                    sl = OH_SLOT[tg]
                    nc.tensor.matmul(
                        ps[:, u * TW:(u + 1) * TW],
                        lhsT=oha[:, l * P:(l + 1) * P],
                        rhs=ohc[:, sl * TW:(sl + 1) * TW],
                        start=False, stop=True)
                if ch == direct_ch:
                    cps["ps_direct"] = ps
                    if emit_direct:
                        nc.vector.tensor_reduce(
                            minb[:, l:l + 1], ps[:], op=MIN, axis=AXX)
                else:
                    cp = cpool.tile([P, CW], F16, tag="cp")
                    nc.scalar.copy(cp[:], ps[:])
                    cps[ch] = cp
            return cps

        def emit_block_tree(l, cps):
            """fp16 min tree over the three copied chunks + masked-pair max."""
            ka, kb, kc = sorted(k for k in cps if k != "ps_direct")
            t1 = t1pool.tile([P, CW], F16, tag="t1")
            nc.vector.tensor_tensor(out=t1[:], in0=cps[ka][:], in1=cps[kb][:], op=MIN)
            t1b = t1pool.tile([P, CW], F16, tag="t1")
            nc.vector.tensor_tensor(out=t1b[:], in0=t1[:], in1=cps[kc][:], op=MIN)
            t2 = t2pool.tile([P, CW // 2], F16, tag="t2")
            nc.vector.tensor_tensor(out=t2[:], in0=t1b[:, 0:1024], in1=t1b[:, 1024:2048], op=MIN)
            t3 = t3pool.tile([P, CW // 4], F16, tag="t3")
            nc.vector.tensor_tensor(out=t3[:], in0=t2[:, 0:512], in1=t2[:, 512:1024], op=MIN)
            nc.vector.tensor_reduce(minb[:, NBLK + l:NBLK + l + 1], t3[:], op=MIN, axis=AXX)
            if l >= 2:
                # same-label window of block l is statically inside
                # cols [128(l-2), 128(l-2)+640) of the c0 copy (group size
                # <= 256); extra diff-label cols can never win the max.
                w0 = P * (l - 2)
                win = cps[0][:, w0:w0 + 640]
                mx = mxpool.tile([P, 320], F16, tag="mx")
                nc.vector.tensor_tensor(out=mx[:], in0=cps[0][:, w0:w0 + 320],
                                        in1=cps[0][:, w0 + 320:w0 + 640], op=MAX)
                nc.vector.tensor_reduce(fin2[:, NBLK + l:NBLK + l + 1], mx[:], op=MAX, axis=AXX)
            else:
                ta, tb = MASKED_TILES[l]
                ca, oa = CP_SLICE[ta]
                cb, ob = CP_SLICE[tb]
                mx = mxpool.tile([P, TW], F16, tag="mx")
                nc.vector.tensor_tensor(out=mx[:], in0=cps[ca][:, oa:oa + TW],
                                        in1=cps[cb][:, ob:ob + TW], op=MAX)
                nc.vector.tensor_reduce(fin2[:, NBLK + l:NBLK + l + 1], mx[:], op=MAX, axis=AXX)

        # software pipeline: tree for block l-1 is emitted after block l's
        # matmuls+drains so the DVE FIFO frees PSUM slots promptly.
        prev = None
        for l in range(NBLK):
            # last block: make c3 the direct chunk so the final tree does not
            # serialize behind the last ACT copy (its masked tiles are in c0),
            # and emit its direct reduce AFTER block 6's tree so the DVE FIFO
            # is not head-blocked waiting on the last matmuls.
            last = l == NBLK - 1
            cps = emit_block_mms(l, direct_ch=1, emit_direct=not last)
            if prev is not None:
                emit_block_tree(l - 1, prev)
            if last:
                nc.vector.tensor_reduce(
                    minb[:, l:l + 1], cps["ps_direct"][:], op=MIN, axis=AXX)
            prev = cps
        emit_block_tree(NBLK - 1, prev)

        per_all = fpool.tile([P, NBLK], F32, tag="per")
        nc.vector.tensor_tensor(out=fin2[:, 0:NBLK], in0=minb[:, 0:NBLK],
                                in1=minb[:, NBLK:2 * NBLK], op=MIN)
        d2b = fpool.tile([P, 2 * NBLK], F32, tag="d2b")
        nc.vector.tensor_add(d2b[:], fin2[:], bias16[:])
        nc.vector.tensor_scalar(out=d2b[:], in0=d2b[:], scalar1=0.0, scalar2=None,
                                op0=MAX)
        dr = fpool.tile([P, 2 * NBLK], F32, tag="dr")
        nc.scalar.activation(dr[:], d2b[:], mybir.ActivationFunctionType.Sqrt)
        df = fpool.tile([P, NBLK], F32, tag="df")
        nc.vector.tensor_sub(df[:], dr[:, NBLK:2 * NBLK], dr[:, 0:NBLK])
        nc.vector.tensor_scalar(out=per_all[:], in0=df[:],
                                scalar1=MARGIN, scalar2=0.0,
                                op0=mybir.AluOpType.add, op1=MAX)
        nc.sync.dma_start(out_d.ap()[:, :], per_all[:])
    return nc


def _reference_fallback(embeddings, labels):
    x = embeddings / np.maximum(
        np.sqrt((embeddings * embeddings).sum(1, keepdims=True)), 1e-12)
    sq = (x * x).sum(1)
    d2 = sq[:, None] + sq[None, :] - 2.0 * (x @ x.T)
    dist = np.sqrt(np.maximum(d2, 0.0))
    same = labels[:, None] == labels[None, :]
    eye = np.eye(len(labels), dtype=bool)
    pos, neg = same & ~eye, ~same
    d_ap = np.where(pos, dist, -np.inf).max(1)
    d_an = np.where(neg, dist, np.inf).min(1)
    valid = pos.any(1) & neg.any(1)
    per = np.maximum(d_ap - d_an + MARGIN, 0.0)
    per = np.where(valid, per, 0.0)
    nz = (per > 0).sum()
    return np.float32(per.sum() / max(nz, 1)) if nz > 0 else np.float32(0.0)


def kernel(embeddings: np.ndarray, labels: np.ndarray) -> np.ndarray:
    global LAST_RESULTS
    emb = np.asarray(embeddings, dtype=np.float32)
    lab = np.asarray(labels).reshape(-1)

    counts = np.bincount(lab.astype(np.int64) - lab.min())
    if emb.shape != (N, D) or counts.max() > 256 or len(np.unique(lab)) < 2:
        return np.array(_reference_fallback(emb, lab), dtype=np.float32)

    norms = np.sqrt((emb * emb).sum(1, keepdims=True, dtype=np.float32))
    xn = emb / np.maximum(norms, np.float32(1e-12))

    perm = np.argsort(lab, kind="stable")
    xs = xn[perm]
    ls = lab[perm]
    # bf16-rounded embeddings: device matmuls see exactly these values
    xsb16 = xs.astype(BF)
    xsb = xsb16.astype(np.float32)
    ss = (xsb * xsb).sum(1, dtype=np.float32)

    # map labels to dense 0..63 codes for the one-hot
    uniq = np.unique(ls)
    code = np.searchsorted(uniq, ls).astype(np.int64)
    assert len(uniq) <= 64

    _install_wait_split_patch()
    _ensure_ntff_hook()
    nc = _build_nc()

    in_maps = []
    for c in range(NCORES):
        lo = c * M
        rot = np.roll(np.arange(N), -lo)            # local col j -> sorted row
        xt = np.ascontiguousarray(xsb16[rot].T)     # [128, 8192] bf16
        xm2a = np.ascontiguousarray((-2.0 * xsb[lo:lo + M]).T).astype(BF)
        slab = np.concatenate([rot[N - TW:], rot[:3 * TW]])   # local cols 7680:8192 + 0:1536
        ohc = (code[slab][None, :] == np.arange(64)[:, None]).astype(BF)
        oha = (BIG * (code[lo:lo + M][None, :] == np.arange(64)[:, None])).astype(BF)
        bmin = np.ascontiguousarray((1.0 + ss[lo:lo + M]).reshape(NBLK, P).T.astype(np.float32))
        bmax = (bmin - np.float32(BIG)).astype(np.float32)
        in_maps.append({"xt": xt, "xm2a": xm2a,
                        "ohb": np.concatenate([oha, ohc], axis=1),
                        "bias16": np.concatenate([bmin, bmax], axis=1)})

    res = run_bass_kernel_spmd(nc, in_maps, core_ids=list(range(NCORES)))
    LAST_RESULTS = res

    per = np.concatenate(
        [res.results[c]["per_out"].T.reshape(M) for c in range(NCORES)])
    nz = int((per > 0).sum())
    if nz == 0:
        return np.array(0.0, dtype=np.float32)
    return np.array(np.float32(per.sum(dtype=np.float64) / nz), dtype=np.float32)


if __name__ == "__main__":
    # quick native compile smoke (no device run)
    from concourse import bass_utils
    import tempfile
    _install_wait_split_patch()
    nc = _build_nc()
    td = tempfile.mkdtemp(prefix="tripletk_")
    print(bass_utils.compile_bass_kernel(nc, td))


# revision 39
# speedup vs baseline: 1.0192x; 1.0192x over previous
"""HardNegTripletMarginLoss on 8 Trainium2 NeuronCores (Bass/Tile).

Strategy (anchors row-sharded across 8 cores, embeddings replicated):
  - Host: normalize rows (as reference), stable-sort rows by label, and give
    each core a column-ROTATED copy of Xn^T so the core's own anchor block
    sits at local columns [0, 1024). With sorted labels every anchor's
    same-label columns then fall inside 2 statically-known 512-wide column
    tiles per anchor block -- identical tile indices on every core, so one
    SPMD program serves all 8 cores.
  - Device (per 128-anchor block, 4 PSUM chunks of 2048 = 16 col-tiles):
    PSUM tile = -2*G via bf16 matmuls (full PE rate) and, on the two
    "masked" tiles per block, += 8*same via a one-hot (K=64) matmul.
    Chunk c1 is min-reduced directly from PSUM on the Vector engine;
    chunks c0/c2/c3 are copied PSUM->SBUF as fp16 by the Scalar engine and
    min-reduced on Vector with a 2x-mode tensor_tensor tree. The two masked
    tiles (always inside c0/c3) get a fused max pair + reduce for the
    hardest-positive.  d2 = s_i + s_j - 2G with s_j ~= 1; s_i applied
    exactly via a per-partition bias before sqrt.
  - Host: gather per-anchor losses, loss = sum(per)/count(per>0).

This walrus build rejects instructions carrying >1 sync wait, so
Bass.to_json_bytes is wrapped to split multi-wait instructions into
single-wait Drain carriers on the same engine.
"""

import json
import os
import sys
import types
import ctypes

for _p in ("/opt/trn_rl_repo", "/root/.axon_site/_ro/trn_rl_repo"):
    if os.path.isdir(_p) and _p not in sys.path:
        sys.path.append(_p)

import numpy as np
import ml_dtypes
import concourse.bass as bass
import concourse.tile as tile
from concourse import mybir
from concourse.bass_utils import run_bass_kernel_spmd
from contextlib import ExitStack

P = 128
N = 8192
D = 128
NCORES = 8
M = N // NCORES            # anchors per core
NBLK = M // P              # anchor blocks per core
TW = 512                   # matmul moving-dim tile
CW = 2048                  # PSUM chunk (4 banks)
NCH = N // CW
TPC = CW // TW
BIG = 8.0
MARGIN = 0.05
F32 = mybir.dt.float32
F16 = mybir.dt.float16
BF16 = mybir.dt.bfloat16
BF = ml_dtypes.bfloat16

# masked (diagonal-window) global col-tile indices per anchor block l
MASKED_TILES = {0: (15, 0), 1: (15, 0), 2: (0, 1), 3: (0, 1),
                4: (0, 1), 5: (0, 1), 6: (1, 2), 7: (1, 2)}
# column slab layout of the one-hot cols input: tile -> slab slot
OH_SLOT = {15: 0, 0: 1, 1: 2, 2: 3}
# fp16-copy slice (chunk, offset) for each maskable tile
CP_SLICE = {15: (3, 1536), 0: (0, 0), 1: (0, 512), 2: (0, 1024)}

LAST_RESULTS = None        # BassKernelResults of the most recent run (for test.py)


def _install_wait_split_patch():
    if getattr(bass.Bass, "_wait_split_patched", False):
        return
    orig = bass.Bass.to_json_bytes

    def patched(self):
        raw = orig(self)
        d = json.loads(raw)
        changed = False
        for fn in d.get("functions", []):
            for blk in fn.get("blocks", []):
                out, k = [], 0
                for ins in blk.get("instructions", []):
                    si = ins.get("sync_info") or {}
                    waits = si.get("on_wait") or []
                    if len(waits) > 1:
                        changed = True
                        for w in waits[:-1]:
                            k += 1
                            out.append({
                                "name": f"{ins['name']}-sw{k}",
                                "opcode": "Drain",
                                "engine": ins["engine"],
                                "ins": [],
                                "outs": [],
                                "is_reset_sema": False,
                                "debug": ins.get("debug", 0),
                                "sync_info": {"on_wait": [w], "on_update": []},
                            })
                        si["on_wait"] = [waits[-1]]
                    out.append(ins)
                blk["instructions"] = out
        return json.dumps(d).encode() if changed else raw

    bass.Bass.to_json_bytes = patched
    bass.Bass._wait_split_patched = True


def _ensure_ntff_hook():
    """Best-effort: restore the axon NTFF profile hook this image dropped."""
    if "antenv.axon_hooks" in sys.modules:
        return
    try:
        lib = ctypes.CDLL("/opt/axon/libaxon_pjrt.so")
        if not hasattr(lib, "axon_start_nrt_profile"):
            return
        from trn_agent_boot.trn_boot import _ntff_profile_via_ctypes
        hook = _ntff_profile_via_ctypes("/opt/axon/libaxon_pjrt.so")
        mod = types.ModuleType("antenv.axon_hooks")
        mod._hook = hook
        mod.get_axon_ntff_profile_hook = lambda: mod._hook
        mod.set_axon_ntff_profile_hook = lambda h: setattr(mod, "_hook", h)
        sys.modules["antenv.axon_hooks"] = mod
        import antenv
        antenv.axon_hooks = mod
    except Exception:
        pass


def _build_nc():
    nc = bass.Bass("TRN2", target_bir_lowering=False, debug=False)
    xt_d = nc.dram_tensor("xt", [P, N], BF16, kind="ExternalInput")
    xa_d = nc.dram_tensor("xm2a", [P, M], BF16, kind="ExternalInput")
    ohb_d = nc.dram_tensor("ohb", [64, M + 4 * TW], BF16, kind="ExternalInput")
    bias_d = nc.dram_tensor("bias16", [P, 2 * NBLK], F32, kind="ExternalInput")
    out_d = nc.dram_tensor("per_out", [P, NBLK], F32, kind="ExternalOutput")

    MIN = mybir.AluOpType.min
    MAX = mybir.AluOpType.max
    AXX = mybir.AxisListType.X

    with tile.TileContext(nc) as tc, ExitStack() as ctx:
        xpool = ctx.enter_context(tc.tile_pool(name="xt", bufs=NCH))
        inpool = ctx.enter_context(tc.tile_pool(name="ins", bufs=1))
        cpool = ctx.enter_context(tc.tile_pool(name="cp", bufs=24))
        t1pool = ctx.enter_context(tc.tile_pool(name="t1", bufs=4))
        t2pool = ctx.enter_context(tc.tile_pool(name="t2", bufs=2))
        t3pool = ctx.enter_context(tc.tile_pool(name="t3", bufs=4))
        mxpool = ctx.enter_context(tc.tile_pool(name="mx", bufs=2))
        ppool = ctx.enter_context(tc.tile_pool(name="psum", bufs=2, space="PSUM"))
        accpool = ctx.enter_context(tc.tile_pool(name="acc", bufs=1))
        fpool = ctx.enter_context(tc.tile_pool(name="fin", bufs=4))

        # DMA order: xa + first xt chunk first so block 0's matmuls start as
        # early as possible; remaining xt chunks follow the small inputs.
        xa = inpool.tile([P, M], BF16, tag="xa")
        nc.sync.dma_start(xa[:], xa_d.ap()[:, :])
        xch = []
        for _i in range(NCH):
            _t = xpool.tile([P, CW], BF16, tag="xch", name=f"xch{_i}")
            xch.append(_t)
        nc.sync.dma_start(xch[0][:], xt_d.ap()[:, 0:CW])
        ohb = inpool.tile([64, M + 4 * TW], BF16, tag="ohb")
        nc.sync.dma_start(ohb[:], ohb_d.ap()[:, :])
        oha = ohb[:, 0:M]
        ohc = ohb[:, M:M + 4 * TW]
        bias16 = inpool.tile([P, 2 * NBLK], F32, tag="bias16")
        nc.sync.dma_start(bias16[:], bias_d.ap()[:, :])
        for ch in range(1, NCH):
            nc.sync.dma_start(xch[ch][:], xt_d.ap()[:, ch * CW:(ch + 1) * CW])

        # minb[:, l] = direct min of chunk c1; minb[:, NBLK+l] = tree min of
        # c0/c2/c3.  fin2 = [merged min | masked max] -- maxes land in the
        # high half so the finale needs no gather.
        minb = accpool.tile([P, 2 * NBLK], F32, tag="minb", name="minb")
        fin2 = accpool.tile([P, 2 * NBLK], F32, tag="fin2", name="fin2")

        # PE pre-warm: matmuls on a memset garbage tile -- no DMA dependency,
        # so they run during the input DMA wait and flip the HAM clock-gate to
        # 2.4 GHz before the first real matmul.
        junk = inpool.tile([P, TW], BF16, tag="junk")
        nc.vector.memset(junk[:], 1.0)
        warm_ps = ppool.tile([P, CW], F32, tag="ps")
        for _ in range(36):
            nc.tensor.matmul(warm_ps[:, 0:P], lhsT=junk[:, 0:P], rhs=junk[:, 0:P],
                             start=True, stop=True)

        def emit_block_mms(l, direct_ch=1, emit_direct=True):
            """PE matmuls + PSUM drains (ACT copies / DVE direct reduce)."""
            lhsT = xa[:, l * P:(l + 1) * P]
            cps = {}
            for ch in range(NCH):
                ps = ppool.tile([P, CW], F32, tag="ps")
                masked_us = []
                for u in range(TPC):
                    tg = ch * TPC + u
                    m = tg in MASKED_TILES[l]
                    if m:
                        masked_us.append((u, tg))
                    nc.tensor.matmul(
                        ps[:, u * TW:(u + 1) * TW],
                        lhsT=lhsT,
                        rhs=xch[ch][:, u * TW:(u + 1) * TW],
                        start=True, stop=not m)
                for u, tg in masked_us:
                    sl = OH_SLOT[tg]
                    nc.tensor.matmul(
                        ps[:, u * TW:(u + 1) * TW],
                        lhsT=oha[:, l * P:(l + 1) * P],
                        rhs=ohc[:, sl * TW:(sl + 1) * TW],
                        start=False, stop=True)
                if ch == direct_ch:
                    cps["ps_direct"] = ps
                    if emit_direct:
                        nc.vector.tensor_reduce(
                            minb[:, l:l + 1], ps[:], op=MIN, axis=AXX)
                else:
                    cp = cpool.tile([P, CW], F16, tag="cp")
                    nc.scalar.copy(cp[:], ps[:])
                    cps[ch] = cp
            return cps

        def emit_block_tree(l, cps):
            """fp16 min tree over the three copied chunks + masked-pair max."""
            ka, kb, kc = sorted(k for k in cps if k != "ps_direct")
            t1 = t1pool.tile([P, CW], F16, tag="t1")
            nc.vector.tensor_tensor(out=t1[:], in0=cps[ka][:], in1=cps[kb][:], op=MIN)
            t1b = t1pool.tile([P, CW], F16, tag="t1")
            nc.vector.tensor_tensor(out=t1b[:], in0=t1[:], in1=cps[kc][:], op=MIN)
            t2 = t2pool.tile([P, CW // 2], F16, tag="t2")
            nc.vector.tensor_tensor(out=t2[:], in0=t1b[:, 0:1024], in1=t1b[:, 1024:2048], op=MIN)
            t3 = t3pool.tile([P, CW // 4], F16, tag="t3")
            nc.vector.tensor_tensor(out=t3[:], in0=t2[:, 0:512], in1=t2[:, 512:1024], op=MIN)
            nc.vector.tensor_reduce(minb[:, NBLK + l:NBLK + l + 1], t3[:], op=MIN, axis=AXX)
            if l >= 2:
                # same-label window of block l is statically inside
                # cols [128(l-2), 128(l-2)+640) of the c0 copy (group size
                # <= 256); extra diff-label cols can never win the max.
                w0 = P * (l - 2)
                win = cps[0][:, w0:w0 + 640]
                mx = mxpool.tile([P, 320], F16, tag="mx")
                nc.vector.tensor_tensor(out=mx[:], in0=cps[0][:, w0:w0 + 320],
                                        in1=cps[0][:, w0 + 320:w0 + 640], op=MAX)
                nc.vector.tensor_reduce(fin2[:, NBLK + l:NBLK + l + 1], mx[:], op=MAX, axis=AXX)
            else:
                ta, tb = MASKED_TILES[l]
                ca, oa = CP_SLICE[ta]
                cb, ob = CP_SLICE[tb]
                mx = mxpool.tile([P, TW], F16, tag="mx")
                nc.vector.tensor_tensor(out=mx[:], in0=cps[ca][:, oa:oa + TW],
                                        in1=cps[cb][:, ob:ob + TW], op=MAX)
                nc.vector.tensor_reduce(fin2[:, NBLK + l:NBLK + l + 1], mx[:], op=MAX, axis=AXX)

        # software pipeline: tree for block l-1 is emitted after block l's
        # matmuls+drains so the DVE FIFO frees PSUM slots promptly.
        prev = None
        for l in range(NBLK):
            # last block: make c3 the direct chunk so the final tree does not
            # serialize behind the last ACT copy (its masked tiles are in c0),
            # and emit its direct reduce AFTER block 6's tree so the DVE FIFO
            # is not head-blocked waiting on the last matmuls.
            last = l == NBLK - 1
            cps = emit_block_mms(l, direct_ch=1, emit_direct=not last)
            if prev is not None:
                emit_block_tree(l - 1, prev)
            if last:
                nc.vector.tensor_reduce(
                    minb[:, l:l + 1], cps["ps_direct"][:], op=MIN, axis=AXX)
            prev = cps
        emit_block_tree(NBLK - 1, prev)

        per_all = fpool.tile([P, NBLK], F32, tag="per")
        nc.vector.tensor_tensor(out=fin2[:, 0:NBLK], in0=minb[:, 0:NBLK],
                                in1=minb[:, NBLK:2 * NBLK], op=MIN)
        d2b = fpool.tile([P, 2 * NBLK], F32, tag="d2b")
        nc.vector.tensor_add(d2b[:], fin2[:], bias16[:])
        nc.vector.tensor_scalar(out=d2b[:], in0=d2b[:], scalar1=0.0, scalar2=None,
                                op0=MAX)
        dr = fpool.tile([P, 2 * NBLK], F32, tag="dr")
        nc.scalar.activation(dr[:], d2b[:], mybir.ActivationFunctionType.Sqrt)
        df = fpool.tile([P, NBLK], F32, tag="df")
        nc.vector.tensor_sub(df[:], dr[:, NBLK:2 * NBLK], dr[:, 0:NBLK])
        nc.vector.tensor_scalar(out=per_all[:], in0=df[:],
                                scalar1=MARGIN, scalar2=0.0,
                                op0=mybir.AluOpType.add, op1=MAX)
        nc.sync.dma_start(out_d.ap()[:, :], per_all[:])
    return nc


def _reference_fallback(embeddings, labels):
    x = embeddings / np.maximum(
        np.sqrt((embeddings * embeddings).sum(1, keepdims=True)), 1e-12)
    sq = (x * x).sum(1)
    d2 = sq[:, None] + sq[None, :] - 2.0 * (x @ x.T)
    dist = np.sqrt(np.maximum(d2, 0.0))
    same = labels[:, None] == labels[None, :]
    eye = np.eye(len(labels), dtype=bool)
    pos, neg = same & ~eye, ~same
    d_ap = np.where(pos, dist, -np.inf).max(1)
    d_an = np.where(neg, dist, np.inf).min(1)
    valid = pos.any(1) & neg.any(1)
    per = np.maximum(d_ap - d_an + MARGIN, 0.0)
    per = np.where(valid, per, 0.0)
    nz = (per > 0).sum()
    return np.float32(per.sum() / max(nz, 1)) if nz > 0 else np.float32(0.0)


def kernel(embeddings: np.ndarray, labels: np.ndarray) -> np.ndarray:
    global LAST_RESULTS
    emb = np.asarray(embeddings, dtype=np.float32)
    lab = np.asarray(labels).reshape(-1)

    counts = np.bincount(lab.astype(np.int64) - lab.min())
    if emb.shape != (N, D) or counts.max() > 256 or len(np.unique(lab)) < 2:
        return np.array(_reference_fallback(emb, lab), dtype=np.float32)

    norms = np.sqrt((emb * emb).sum(1, keepdims=True, dtype=np.float32))
    xn = emb / np.maximum(norms, np.float32(1e-12))

    perm = np.argsort(lab, kind="stable")
    xs = xn[perm]
    ls = lab[perm]
    # bf16-rounded embeddings: device matmuls see exactly these values
    xsb16 = xs.astype(BF)
    xsb = xsb16.astype(np.float32)
    ss = (xsb * xsb).sum(1, dtype=np.float32)

    # map labels to dense 0..63 codes for the one-hot
    uniq = np.unique(ls)
    code = np.searchsorted(uniq, ls).astype(np.int64)
    assert len(uniq) <= 64

    _install_wait_split_patch()
    _ensure_ntff_hook()
    nc = _build_nc()

    in_maps = []
    for c in range(NCORES):
        lo = c * M
        rot = np.roll(np.arange(N), -lo)            # local col j -> sorted row
        xt = np.ascontiguousarray(xsb16[rot].T)     # [128, 8192] bf16
        xm2a = np.ascontiguousarray((-2.0 * xsb[lo:lo + M]).T).astype(BF)
        slab = np.concatenate([rot[N - TW:], rot[:3 * TW]])   # local cols 7680:8192 + 0:1536
        ohc = (code[slab][None, :] == np.arange(64)[:, None]).astype(BF)
        oha = (BIG * (code[lo:lo + M][None, :] == np.arange(64)[:, None])).astype(BF)
        bmin = np.ascontiguousarray((1.0 + ss[lo:lo + M]).reshape(NBLK, P).T.astype(np.float32))
        bmax = (bmin - np.float32(BIG)).astype(np.float32)
        in_maps.append({"xt": xt, "xm2a": xm2a,
                        "ohb": np.concatenate([oha, ohc], axis=1),
                        "bias16": np.concatenate([bmin, bmax], axis=1)})

    res = run_bass_kernel_spmd(nc, in_maps, core_ids=list(range(NCORES)))
    LAST_RESULTS = res

    per = np.concatenate(
        [res.results[c]["per_out"].T.reshape(M) for c in range(NCORES)])
    nz = int((per > 0).sum())
    if nz == 0:
        return np.array(0.0, dtype=np.float32)
    return np.array(np.float32(per.sum(dtype=np.float64) / nz), dtype=np.float32)


if __name__ == "__main__":
    # quick native compile smoke (no device run)
    from concourse import bass_utils
    import tempfile
    _install_wait_split_patch()
    nc = _build_nc()
    td = tempfile.mkdtemp(prefix="tripletk_")
    print(bass_utils.compile_bass_kernel(nc, td))
